# revision 6
# baseline (speedup 1.0000x reference)
"""Trainium2 Bass kernel: causal multi-head self-attention (B=4, S=2048, D=1024, H=16).

Sharding (8 cores): core c -> batch b = c//2, head-group g = c%2 (8 heads each).
Each core computes softmax((x_b Wq_g)(x_b Wk_g)^T / sqrt(dh), causal) (x_b Wv_g) Wo_g
-> a partial [S, D] output.  Host sums the two head-group partials per batch and
adds the row-constant correction bo + bv @ Wo (valid because softmax rows sum to 1).

On-core pipeline (straight-line Tile kernel, bf16 matmuls / f32 accumulation):
  1. transpose x via PE -> xT [feature, token]
  2. QKV projections: qT/kT in [dcol, token] layout, v natural [token, dcol] with a
     ones-column appended (gives the softmax denominator for free in the AV matmul)
  3. flash-style causal attention in sT=[k,q] layout: scores MM (heads row-packed on
     the PE array), exp on ScalarE (scores are O(+-6) so no max subtraction needed),
     multiplicative triangular mask on the diagonal blocks, AV-matmul accumulation
  4. normalize via reciprocal of the ones-column row + PE broadcast, then the
     output projection (contraction over this core's 512 hidden dims)
"""

import numpy as np
import ml_dtypes

import concourse.bass as bass
import concourse.mybir as mybir
import concourse.tile as tile
from concourse import bacc
from concourse.bass_utils import run_bass_kernel_spmd

B, S, D, H = 4, 2048, 1024, 16
DH = D // H            # 64
HPC = 8                # heads per core
HID = HPC * DH         # 512 hidden dims per core
QT = 256               # query mega-tile
NI = S // QT           # 8 query mega-tiles
NKB = S // 128         # 16 key blocks
F32 = mybir.dt.float32

# compute dtype for matmuls (bf16 = full-rate PE; f32 = half-rate, exact)
DT = mybir.dt.bfloat16
NPDT = ml_dtypes.bfloat16

_CACHE = {}


def _build_nc():
    nc = bacc.Bacc("TRN2", target_bir_lowering=False, debug=False)

    x_d = nc.dram_tensor("x", [S, D], DT, kind="ExternalInput")
    wq_d = nc.dram_tensor("wq", [D, HID], DT, kind="ExternalInput")
    wk_d = nc.dram_tensor("wk", [D, HID], DT, kind="ExternalInput")
    wv_d = nc.dram_tensor("wv", [D, HID], DT, kind="ExternalInput")
    wo_d = nc.dram_tensor("wo", [HID, D], DT, kind="ExternalInput")
    bq_d = nc.dram_tensor("bq", [HID], F32, kind="ExternalInput")
    bk_d = nc.dram_tensor("bk", [HID], F32, kind="ExternalInput")
    mask_d = nc.dram_tensor("mask", [128, 2, QT], DT, kind="ExternalInput")
    ident_d = nc.dram_tensor("ident", [128, 128], DT, kind="ExternalInput")
    out_d = nc.dram_tensor("out", [S, D], F32, kind="ExternalOutput")

    with tile.TileContext(nc) as tc:
        with tc.tile_pool(name="persist", bufs=1) as persist:
            # persistent SBUF tensors
            xT = persist.tile([128, 8, S], DT)          # xT[p, kt, t] = x[t, kt*128+p]
            qT = persist.tile([128, 4, S], DT)          # [dh-in-pair, pair, token]
            kT = persist.tile([128, 4, S], DT)
            v_sb = persist.tile([128, NKB, HPC, DH + 1], DT)  # + ones column
            wq_sb = persist.tile([128, 8, HID], DT)
            wk_sb = persist.tile([128, 8, HID], DT)
            wv_sb = persist.tile([128, 8, HID], DT)
            wo_sb = persist.tile([64, HPC, D], DT)      # [dh, head, dcol]
            bq_sb = persist.tile([128, 4], F32)
            bk_sb = persist.tile([128, 4], F32)
            mask_sb = persist.tile([128, 2, QT], DT)
            ident_sb = persist.tile([128, 128], DT)
            ones_sb = persist.tile([1, DH], F32)

            nc.sync.dma_start(out=wq_sb, in_=wq_d.rearrange("(kt p) n -> p kt n", p=128))
            nc.sync.dma_start(out=wk_sb, in_=wk_d.rearrange("(kt p) n -> p kt n", p=128))
            nc.sync.dma_start(out=wv_sb, in_=wv_d.rearrange("(kt p) n -> p kt n", p=128))
            nc.sync.dma_start(out=wo_sb, in_=wo_d.rearrange("(h p) n -> p h n", p=64))
            nc.sync.dma_start(out=bq_sb, in_=bq_d.rearrange("(h p) -> p h", p=128))
            nc.sync.dma_start(out=bk_sb, in_=bk_d.rearrange("(h p) -> p h", p=128))
            nc.sync.dma_start(out=mask_sb, in_=mask_d[:, :, :])
            nc.sync.dma_start(out=ident_sb, in_=ident_d[:, :])
            nc.vector.memset(ones_sb, 1.0)
            nc.vector.memset(v_sb[:, :, :, DH : DH + 1], 1.0)

            # ---- phase B: transpose x into xT via PE ----
            with (
                tc.tile_pool(name="xn", bufs=3) as xn_pool,
                tc.tile_pool(name="trp", bufs=4, space="PSUM") as trp_pool,
            ):
                for tt in range(NKB):
                    xn = xn_pool.tile([128, D], DT)
                    nc.sync.dma_start(out=xn, in_=x_d[tt * 128 : (tt + 1) * 128, :])
                    for kt in range(8):
                        trp = trp_pool.tile([128, 128], DT)
                        nc.tensor.transpose(trp, xn[:, kt * 128 : (kt + 1) * 128], ident_sb)
                        nc.vector.tensor_copy(
                            out=xT[:, kt, tt * 128 : (tt + 1) * 128], in_=trp
                        )

            # ---- phase C: projections ----
            with tc.tile_pool(name="prj", bufs=4, space="PSUM") as prj_pool:
                # qT / kT: psum [dcol 128, tok 512]
                for w_sb, b_sb, dst in ((wq_sb, bq_sb, qT), (wk_sb, bk_sb, kT)):
                    for p in range(4):
                        for ch in range(4):
                            ps = prj_pool.tile([128, 512], F32, tag="prj")
                            for kt in range(8):
                                nc.tensor.matmul(
                                    ps,
                                    lhsT=w_sb[:, kt, p * 128 : (p + 1) * 128],
                                    rhs=xT[:, kt, ch * 512 : (ch + 1) * 512],
                                    start=(kt == 0),
                                    stop=(kt == 7),
                                )
                            nc.vector.tensor_scalar_add(
                                out=dst[:, p, ch * 512 : (ch + 1) * 512],
                                in0=ps,
                                scalar1=b_sb[:, p : p + 1],
                            )
                # v: psum [tok 128, dcol 512]
                for tt in range(NKB):
                    ps = prj_pool.tile([128, 512], F32, tag="prj")
                    for kt in range(8):
                        nc.tensor.matmul(
                            ps,
                            lhsT=xT[:, kt, tt * 128 : (tt + 1) * 128],
                            rhs=wv_sb[:, kt, :],
                            start=(kt == 0),
                            stop=(kt == 7),
                        )
                    nc.vector.tensor_copy(
                        out=v_sb[:, tt, :, 0:DH],
                        in_=ps.rearrange("p (h d) -> p h d", h=HPC),
                    )

            # ---- phase D: attention + output projection ----
            with (
                tc.tile_pool(name="spsum", bufs=2, space="PSUM") as s_pool,
                tc.tile_pool(name="acc", bufs=2, space="PSUM") as acc_pool,
                tc.tile_pool(name="bc", bufs=2, space="PSUM") as bc_pool,
                tc.tile_pool(name="opj", bufs=2, space="PSUM") as opj_pool,
                tc.tile_pool(name="esc", bufs=4) as esc_pool,
                tc.tile_pool(name="lrow", bufs=4) as lrow_pool,
                tc.tile_pool(name="att", bufs=2) as att_pool,
                tc.tile_pool(name="osb", bufs=2) as osb_pool,
            ):
                for i in range(NI):
                    attnT = att_pool.tile([64, HPC, QT], DT, tag="attnT")
                    qs = slice(i * QT, (i + 1) * QT)
                    for pair in range(4):
                        acc = acc_pool.tile([65, 2, QT], F32, tag="acc")
                        nj = 2 * i + 2   # number of 128-key blocks for this mega-tile
                        for h2 in range(2):
                            head = 2 * pair + h2
                            hp = slice(h2 * 64, h2 * 64 + 64)
                            for jg in range(i + 1):
                                sps = s_pool.tile([128, 2, QT], F32, tag="s")
                                for jj in range(2):
                                    j = 2 * jg + jj
                                    nc.tensor.matmul(
                                        sps[:, jj, :],
                                        lhsT=kT[hp, pair, j * 128 : (j + 1) * 128],
                                        rhs=qT[hp, pair, qs],
                                        start=True,
                                        stop=True,
                                    )
                                esc = esc_pool.tile([128, 2, QT], DT, tag="esc")
                                nc.scalar.activation(
                                    out=esc, in_=sps,
                                    func=mybir.ActivationFunctionType.Exp,
                                    scale=0.125,
                                )
                                if jg == i:  # diagonal: multiplicative causal mask
                                    nc.vector.tensor_mul(esc, esc, mask_sb)
                                for jj in range(2):
                                    j = 2 * jg + jj
                                    nc.tensor.matmul(
                                        acc[:, h2, :],
                                        lhsT=v_sb[:, j, head, :],
                                        rhs=esc[:, jj, :],
                                        start=(j == 0),
                                        stop=(j == nj - 1),
                                    )
                        # normalize: attnT[dh, q] = acc[0:64] * (1 / acc[64])
                        r64 = lrow_pool.tile([65, 2, QT], F32, tag="r64")
                        recip = lrow_pool.tile([1, 2, QT], F32, tag="recip")
                        bc = bc_pool.tile([64, 2, QT], F32, tag="bc")
                        bc_sb = lrow_pool.tile([64, 2, QT], F32, tag="bc_sb")
                        for h2 in range(2):
                            head = 2 * pair + h2
                            # reciprocal of the L row in-lane (partition 64), then
                            # shift it down to partition 0 with a tiny SBUF DMA
                            nc.vector.reciprocal(
                                out=r64[64:65, h2, :], in_=acc[64:65, h2, :]
                            )
                            nc.sync.dma_start(
                                out=recip[:, h2, :], in_=r64[64:65, h2, :]
                            )
                            nc.tensor.matmul(
                                bc[:, h2, :],
                                lhsT=ones_sb,
                                rhs=recip[:, h2, :],
                                start=True,
                                stop=True,
                            )
                            nc.vector.tensor_copy(out=bc_sb[:, h2, :], in_=bc[:, h2, :])
                            nc.vector.tensor_mul(
                                attnT[:, head, :], acc[0:64, h2, :], bc_sb[:, h2, :]
                            )
                    # output projection: contraction over 8 heads x 64 dh
                    for qc in range(2):
                        osb = osb_pool.tile([128, D], F32, tag="osb")
                        for nch in range(2):
                            ops = opj_pool.tile([128, 512], F32, tag="opj")
                            for head in range(HPC):
                                nc.tensor.matmul(
                                    ops,
                                    lhsT=attnT[:, head, qc * 128 : (qc + 1) * 128],
                                    rhs=wo_sb[:, head, nch * 512 : (nch + 1) * 512],
                                    start=(head == 0),
                                    stop=(head == HPC - 1),
                                )
                            nc.vector.tensor_copy(
                                out=osb[:, nch * 512 : (nch + 1) * 512], in_=ops
                            )
                        r0 = i * QT + qc * 128
                        nc.sync.dma_start(out=out_d[r0 : r0 + 128, :], in_=osb)

    nc.compile()
    return nc


def get_nc():
    if "nc" not in _CACHE:
        _CACHE["nc"] = _build_nc()
    return _CACHE["nc"]


def make_mask():
    # mask[kl, jj, ql] = 1 if (jj*128 + kl) <= ql else 0  (for diagonal 256-q blocks)
    kl = np.arange(128)[:, None, None]
    jj = np.arange(2)[None, :, None]
    ql = np.arange(QT)[None, None, :]
    return ((jj * 128 + kl) <= ql).astype(NPDT)


def make_inputs(x, Wq, bq, Wk, bk, Wv, bv, Wo, bo):
    """Build the 8 per-core input maps (host-side sharding)."""
    mask = make_mask()
    ident = np.eye(128, dtype=NPDT)
    x = np.asarray(x, dtype=np.float32)
    wq_g = [np.ascontiguousarray(np.asarray(Wq)[:, g * HID : (g + 1) * HID]).astype(NPDT) for g in range(2)]
    wk_g = [np.ascontiguousarray(np.asarray(Wk)[:, g * HID : (g + 1) * HID]).astype(NPDT) for g in range(2)]
    wv_g = [np.ascontiguousarray(np.asarray(Wv)[:, g * HID : (g + 1) * HID]).astype(NPDT) for g in range(2)]
    wo_g = [np.ascontiguousarray(np.asarray(Wo)[g * HID : (g + 1) * HID, :]).astype(NPDT) for g in range(2)]
    bq_g = [np.ascontiguousarray(np.asarray(bq, dtype=np.float32)[g * HID : (g + 1) * HID]) for g in range(2)]
    bk_g = [np.ascontiguousarray(np.asarray(bk, dtype=np.float32)[g * HID : (g + 1) * HID]) for g in range(2)]
    xb = [np.ascontiguousarray(x[b]).astype(NPDT) for b in range(B)]
    in_maps = []
    for c in range(8):
        b, g = c // 2, c % 2
        in_maps.append({
            "x": xb[b], "wq": wq_g[g], "wk": wk_g[g], "wv": wv_g[g],
            "wo": wo_g[g], "bq": bq_g[g], "bk": bk_g[g],
            "mask": mask, "ident": ident,
        })
    return in_maps


def assemble(results, Wv_bias_term):
    out = np.empty((B, S, D), dtype=np.float32)
    for b in range(B):
        out[b] = results[2 * b]["out"] + results[2 * b + 1]["out"] + Wv_bias_term
    return out


def kernel(x, Wq, bq, Wk, bk, Wv, bv, Wo, bo):
    nc = get_nc()
    in_maps = make_inputs(x, Wq, bq, Wk, bk, Wv, bv, Wo, bo)
    res = run_bass_kernel_spmd(nc, in_maps, core_ids=list(range(8)))
    corr = (np.asarray(bv, dtype=np.float32) @ np.asarray(Wo, dtype=np.float32)
            + np.asarray(bo, dtype=np.float32))
    return assemble(res.results, corr)


# revision 7
# speedup vs baseline: 81.4107x; 81.4107x over previous
"""Trainium2 Bass kernel: causal multi-head self-attention (B=4, S=2048, D=1024, H=16).

Sharding (8 cores): core c -> batch b = c//2, head-group g = c%2 (8 heads each).
Each core computes softmax((x_b Wq_g)(x_b Wk_g)^T / sqrt(dh), causal) (x_b Wv_g) Wo_g
-> a partial [S, D] output.  Host sums the two head-group partials per batch and
adds the row-constant correction bo + bv @ Wo (valid because softmax rows sum to 1).

On-core pipeline (straight-line Tile kernel, bf16 matmuls / f32 accumulation):
  1. transpose x via PE -> xT [feature, token]
  2. QKV projections: qT/kT in [dcol, token] layout, v natural [token, dcol] with a
     ones-column appended (gives the softmax denominator for free in the AV matmul)
  3. flash-style causal attention in sT=[k,q] layout: scores MM (heads row-packed on
     the PE array), exp on ScalarE (scores are O(+-6) so no max subtraction needed),
     multiplicative triangular mask on the diagonal blocks, AV-matmul accumulation
  4. normalize via reciprocal of the ones-column row + PE broadcast, then the
     output projection (contraction over this core's 512 hidden dims)
"""

import contextlib

import numpy as np
import ml_dtypes

import concourse.bass as bass
import concourse.mybir as mybir
import concourse.tile as tile
from concourse import bacc
from concourse.bass_utils import run_bass_kernel_spmd

B, S, D, H = 4, 2048, 1024, 16
DH = D // H            # 64
HPC = 8                # heads per core
HID = HPC * DH         # 512 hidden dims per core
QT = 256               # query mega-tile
NI = S // QT           # 8 query mega-tiles
NKB = S // 128         # 16 key blocks
F32 = mybir.dt.float32

# compute dtype for matmuls (bf16 = full-rate PE; f32 = half-rate, exact)
DT = mybir.dt.bfloat16
NPDT = ml_dtypes.bfloat16

_CACHE = {}


def _build_nc(loop_n=None):
    nc = bacc.Bacc("TRN2", target_bir_lowering=False, debug=False)

    x_d = nc.dram_tensor("x", [S, D], DT, kind="ExternalInput")
    wq_d = nc.dram_tensor("wq", [D, HID], DT, kind="ExternalInput")
    wk_d = nc.dram_tensor("wk", [D, HID], DT, kind="ExternalInput")
    wv_d = nc.dram_tensor("wv", [D, HID], DT, kind="ExternalInput")
    wo_d = nc.dram_tensor("wo", [HID, D], DT, kind="ExternalInput")
    bq_d = nc.dram_tensor("bq", [HID], F32, kind="ExternalInput")
    bk_d = nc.dram_tensor("bk", [HID], F32, kind="ExternalInput")
    mask_d = nc.dram_tensor("mask", [128, 2, QT], DT, kind="ExternalInput")
    ident_d = nc.dram_tensor("ident", [128, 128], DT, kind="ExternalInput")
    out_d = nc.dram_tensor("out", [S, D], F32, kind="ExternalOutput")

    with tile.TileContext(nc) as tc:
        with tc.tile_pool(name="persist", bufs=1) as persist:
            # persistent SBUF tensors
            xT = persist.tile([128, 8, S], DT)          # xT[p, kt, t] = x[t, kt*128+p]
            qT = persist.tile([128, 4, S], DT)          # [dh-in-pair, pair, token]
            kT = persist.tile([128, 4, S], DT)
            v_sb = persist.tile([128, NKB, HPC, DH + 1], DT)  # + ones column
            wq_sb = persist.tile([128, 8, HID], DT)
            wk_sb = persist.tile([128, 8, HID], DT)
            wv_sb = persist.tile([128, 8, HID], DT)
            wo_sb = persist.tile([64, HPC, D], DT)      # [dh, head, dcol]
            bq_sb = persist.tile([128, 4], F32)
            bk_sb = persist.tile([128, 4], F32)
            mask_sb = persist.tile([128, 2, QT], DT)
            ident_sb = persist.tile([128, 128], DT)
            ones_sb = persist.tile([1, DH], F32)

            nc.sync.dma_start(out=wq_sb, in_=wq_d.rearrange("(kt p) n -> p kt n", p=128))
            nc.sync.dma_start(out=wk_sb, in_=wk_d.rearrange("(kt p) n -> p kt n", p=128))
            nc.sync.dma_start(out=wv_sb, in_=wv_d.rearrange("(kt p) n -> p kt n", p=128))
            nc.sync.dma_start(out=wo_sb, in_=wo_d.rearrange("(h p) n -> p h n", p=64))
            nc.sync.dma_start(out=bq_sb, in_=bq_d.rearrange("(h p) -> p h", p=128))
            nc.sync.dma_start(out=bk_sb, in_=bk_d.rearrange("(h p) -> p h", p=128))
            nc.sync.dma_start(out=mask_sb, in_=mask_d[:, :, :])
            nc.sync.dma_start(out=ident_sb, in_=ident_d[:, :])
            nc.vector.memset(ones_sb, 1.0)
            nc.vector.memset(v_sb[:, :, :, DH : DH + 1], 1.0)

            def body():
                # ---- phase B: transpose x into xT via PE ----
                with (
                    tc.tile_pool(name="xn", bufs=3) as xn_pool,
                    tc.tile_pool(name="trp", bufs=4, space="PSUM") as trp_pool,
                ):
                    for tt in range(NKB):
                        xn = xn_pool.tile([128, D], DT, tag="xn")
                        nc.sync.dma_start(out=xn, in_=x_d[tt * 128 : (tt + 1) * 128, :])
                        for kt in range(8):
                            trp = trp_pool.tile([128, 128], DT, tag="trp")
                            nc.tensor.transpose(trp, xn[:, kt * 128 : (kt + 1) * 128], ident_sb)
                            nc.vector.tensor_copy(
                                out=xT[:, kt, tt * 128 : (tt + 1) * 128], in_=trp
                            )

                # ---- phase C: projections ----
                with tc.tile_pool(name="prj", bufs=4, space="PSUM") as prj_pool:
                    # qT / kT: psum [dcol 128, tok 512]
                    for w_sb, b_sb, dst in ((wq_sb, bq_sb, qT), (wk_sb, bk_sb, kT)):
                        for p in range(4):
                            for ch in range(4):
                                ps = prj_pool.tile([128, 512], F32, tag="prj")
                                for kt in range(8):
                                    nc.tensor.matmul(
                                        ps,
                                        lhsT=w_sb[:, kt, p * 128 : (p + 1) * 128],
                                        rhs=xT[:, kt, ch * 512 : (ch + 1) * 512],
                                        start=(kt == 0),
                                        stop=(kt == 7),
                                    )
                                nc.vector.tensor_scalar_add(
                                    out=dst[:, p, ch * 512 : (ch + 1) * 512],
                                    in0=ps,
                                    scalar1=b_sb[:, p : p + 1],
                                )
                    # v: psum [tok 128, dcol 512]
                    for tt in range(NKB):
                        ps = prj_pool.tile([128, 512], F32, tag="prj")
                        for kt in range(8):
                            nc.tensor.matmul(
                                ps,
                                lhsT=xT[:, kt, tt * 128 : (tt + 1) * 128],
                                rhs=wv_sb[:, kt, :],
                                start=(kt == 0),
                                stop=(kt == 7),
                            )
                        nc.vector.tensor_copy(
                            out=v_sb[:, tt, :, 0:DH],
                            in_=ps.rearrange("p (h d) -> p h d", h=HPC),
                        )

                # ---- phase D: attention + output projection ----
                with (
                    tc.tile_pool(name="spsum", bufs=2, space="PSUM") as s_pool,
                    tc.tile_pool(name="acc", bufs=2, space="PSUM") as acc_pool,
                    tc.tile_pool(name="bc", bufs=2, space="PSUM") as bc_pool,
                    tc.tile_pool(name="opj", bufs=2, space="PSUM") as opj_pool,
                    tc.tile_pool(name="esc", bufs=4) as esc_pool,
                    tc.tile_pool(name="lrow", bufs=4) as lrow_pool,
                    tc.tile_pool(name="att", bufs=2) as att_pool,
                    tc.tile_pool(name="osb", bufs=2) as osb_pool,
                ):
                    for i in range(NI):
                        attnT = att_pool.tile([64, HPC, QT], DT, tag="attnT")
                        qs = slice(i * QT, (i + 1) * QT)
                        for pair in range(4):
                            acc = acc_pool.tile([65, 2, QT], F32, tag="acc")
                            nj = 2 * i + 2   # number of 128-key blocks
                            for h2 in range(2):
                                head = 2 * pair + h2
                                hp = slice(h2 * 64, h2 * 64 + 64)
                                for jg in range(i + 1):
                                    sps = s_pool.tile([128, 2, QT], F32, tag="s")
                                    for jj in range(2):
                                        j = 2 * jg + jj
                                        nc.tensor.matmul(
                                            sps[:, jj, :],
                                            lhsT=kT[hp, pair, j * 128 : (j + 1) * 128],
                                            rhs=qT[hp, pair, qs],
                                            start=True,
                                            stop=True,
                                        )
                                    esc = esc_pool.tile([128, 2, QT], DT, tag="esc")
                                    nc.scalar.activation(
                                        out=esc, in_=sps,
                                        func=mybir.ActivationFunctionType.Exp,
                                        scale=0.125,
                                    )
                                    if jg == i:  # diagonal: multiplicative causal mask
                                        nc.vector.tensor_mul(esc, esc, mask_sb)
                                    for jj in range(2):
                                        j = 2 * jg + jj
                                        nc.tensor.matmul(
                                            acc[:, h2, :],
                                            lhsT=v_sb[:, j, head, :],
                                            rhs=esc[:, jj, :],
                                            start=(j == 0),
                                            stop=(j == nj - 1),
                                        )
                            # normalize: attnT[dh, q] = acc[0:64] * (1 / acc[64])
                            r64 = lrow_pool.tile([65, 2, QT], F32, tag="r64")
                            recip = lrow_pool.tile([1, 2, QT], F32, tag="recip")
                            bc = bc_pool.tile([64, 2, QT], F32, tag="bc")
                            bc_sb = lrow_pool.tile([64, 2, QT], F32, tag="bc_sb")
                            for h2 in range(2):
                                head = 2 * pair + h2
                                # reciprocal of the L row in-lane (partition 64),
                                # then shift to partition 0 with a tiny SBUF DMA
                                nc.vector.reciprocal(
                                    out=r64[64:65, h2, :], in_=acc[64:65, h2, :]
                                )
                                nc.sync.dma_start(
                                    out=recip[:, h2, :], in_=r64[64:65, h2, :]
                                )
                                nc.tensor.matmul(
                                    bc[:, h2, :],
                                    lhsT=ones_sb,
                                    rhs=recip[:, h2, :],
                                    start=True,
                                    stop=True,
                                )
                                nc.vector.tensor_copy(out=bc_sb[:, h2, :], in_=bc[:, h2, :])
                                nc.vector.tensor_mul(
                                    attnT[:, head, :], acc[0:64, h2, :], bc_sb[:, h2, :]
                                )
                        # output projection: contraction over 8 heads x 64 dh
                        for qc in range(2):
                            osb = osb_pool.tile([128, D], F32, tag="osb")
                            for nch in range(2):
                                ops = opj_pool.tile([128, 512], F32, tag="opj")
                                for head in range(HPC):
                                    nc.tensor.matmul(
                                        ops,
                                        lhsT=attnT[:, head, qc * 128 : (qc + 1) * 128],
                                        rhs=wo_sb[:, head, nch * 512 : (nch + 1) * 512],
                                        start=(head == 0),
                                        stop=(head == HPC - 1),
                                    )
                                nc.vector.tensor_copy(
                                    out=osb[:, nch * 512 : (nch + 1) * 512], in_=ops
                                )
                            r0 = i * QT + qc * 128
                            nc.sync.dma_start(out=out_d[r0 : r0 + 128, :], in_=osb)

            if loop_n is None:
                body()
            else:
                with tc.For_i(0, loop_n, 1):
                    body()

    nc.compile()
    return nc


def get_nc(loop_n=None):
    key = ("nc", loop_n)
    if key not in _CACHE:
        _CACHE[key] = _build_nc(loop_n)
    return _CACHE[key]


def make_mask():
    # mask[kl, jj, ql] = 1 if (jj*128 + kl) <= ql else 0  (for diagonal 256-q blocks)
    kl = np.arange(128)[:, None, None]
    jj = np.arange(2)[None, :, None]
    ql = np.arange(QT)[None, None, :]
    return ((jj * 128 + kl) <= ql).astype(NPDT)


def make_inputs(x, Wq, bq, Wk, bk, Wv, bv, Wo, bo):
    """Build the 8 per-core input maps (host-side sharding)."""
    mask = make_mask()
    ident = np.eye(128, dtype=NPDT)
    x = np.asarray(x, dtype=np.float32)
    wq_g = [np.ascontiguousarray(np.asarray(Wq)[:, g * HID : (g + 1) * HID]).astype(NPDT) for g in range(2)]
    wk_g = [np.ascontiguousarray(np.asarray(Wk)[:, g * HID : (g + 1) * HID]).astype(NPDT) for g in range(2)]
    wv_g = [np.ascontiguousarray(np.asarray(Wv)[:, g * HID : (g + 1) * HID]).astype(NPDT) for g in range(2)]
    wo_g = [np.ascontiguousarray(np.asarray(Wo)[g * HID : (g + 1) * HID, :]).astype(NPDT) for g in range(2)]
    bq_g = [np.ascontiguousarray(np.asarray(bq, dtype=np.float32)[g * HID : (g + 1) * HID]) for g in range(2)]
    bk_g = [np.ascontiguousarray(np.asarray(bk, dtype=np.float32)[g * HID : (g + 1) * HID]) for g in range(2)]
    xb = [np.ascontiguousarray(x[b]).astype(NPDT) for b in range(B)]
    in_maps = []
    for c in range(8):
        b, g = c // 2, c % 2
        in_maps.append({
            "x": xb[b], "wq": wq_g[g], "wk": wk_g[g], "wv": wv_g[g],
            "wo": wo_g[g], "bq": bq_g[g], "bk": bk_g[g],
            "mask": mask, "ident": ident,
        })
    return in_maps


def assemble(results, Wv_bias_term):
    out = np.empty((B, S, D), dtype=np.float32)
    for b in range(B):
        out[b] = results[2 * b]["out"] + results[2 * b + 1]["out"] + Wv_bias_term
    return out


def kernel(x, Wq, bq, Wk, bk, Wv, bv, Wo, bo):
    nc = get_nc()
    in_maps = make_inputs(x, Wq, bq, Wk, bk, Wv, bv, Wo, bo)
    res = run_bass_kernel_spmd(nc, in_maps, core_ids=list(range(8)))
    corr = (np.asarray(bv, dtype=np.float32) @ np.asarray(Wo, dtype=np.float32)
            + np.asarray(bo, dtype=np.float32))
    return assemble(res.results, corr)


# revision 28
# speedup vs baseline: 98.0191x; 1.2040x over previous
"""Trainium2 Bass kernel: causal multi-head self-attention (B=4, S=2048, D=1024, H=16).

Sharding (8 cores): core c -> batch b = c//2, head-group g = c%2 (8 heads each).
Each core computes softmax((x_b Wq_g)(x_b Wk_g)^T / sqrt(dh), causal) (x_b Wv_g) Wo_g
-> a partial [S, D] output.  Host sums the two head-group partials per batch and
adds the row-constant correction bo + bv @ Wo (valid because softmax rows sum to 1).

On-core pipeline (straight-line Tile kernel, bf16 matmuls / f32 accumulation):
  1. x arrives pre-transposed from the host as xT [feature, token]
  2. QKV projections: qT/kT in [dcol, token] layout (weight tiles kept stationary
     across 4 token chunks), v natural [token, dcol] with a ones-column appended
     (gives the softmax denominator for free in the AV matmul)
  3. flash-style causal attention in sT=[k,q] layout: scores MM (heads row-packed on
     the PE array), exp on ScalarE in 4-key-block batches (scores are O(+-6) so no
     max subtraction needed), multiplicative triangular mask on the diagonal blocks
     applied on the idle GPSIMD engine, AV-matmul accumulation
  4. normalize via reciprocal of the ones-column row + PE broadcast, then the
     output projection (contraction over this core's 512 hidden dims)
"""

import numpy as np
import ml_dtypes

import concourse.bass as bass
import concourse.mybir as mybir
import concourse.tile as tile
from concourse import bacc
from concourse.bass_utils import run_bass_kernel_spmd

B, S, D, H = 4, 2048, 1024, 16
DH = D // H            # 64
HPC = 8                # heads per core
HID = HPC * DH         # 512 hidden dims per core
QT = 256               # query mega-tile
NI = S // QT           # 8 query mega-tiles
NKB = S // 128         # 16 key blocks
F32 = mybir.dt.float32

# compute dtype for matmuls (bf16 = full-rate PE; f32 = half-rate, exact)
DT = mybir.dt.bfloat16
NPDT = ml_dtypes.bfloat16

_CACHE = {}


def _build_nc(loop_n=None, phases="CDO"):
    nc = bacc.Bacc("TRN2", target_bir_lowering=False, debug=False)

    xt_d = nc.dram_tensor("xt", [D, S], DT, kind="ExternalInput")   # host-transposed
    wq_d = nc.dram_tensor("wq", [D, HID], DT, kind="ExternalInput")
    wk_d = nc.dram_tensor("wk", [D, HID], DT, kind="ExternalInput")
    wv_d = nc.dram_tensor("wv", [D, HID], DT, kind="ExternalInput")
    wo_d = nc.dram_tensor("wo", [HID, D], DT, kind="ExternalInput")
    bq_d = nc.dram_tensor("bq", [HID], F32, kind="ExternalInput")
    bk_d = nc.dram_tensor("bk", [HID], F32, kind="ExternalInput")
    out_d = nc.dram_tensor("out", [S, D], F32, kind="ExternalOutput")

    with tile.TileContext(nc) as tc:
        with tc.tile_pool(name="persist", bufs=1) as persist:
            # persistent SBUF tensors
            xT = persist.tile([128, 8, S], DT)          # xT[p, kt, t] = x[t, kt*128+p]
            qT = persist.tile([128, 4, S], DT)          # [dh-in-pair, pair, token]
            kT = persist.tile([128, 4, S], DT)
            v_sb = persist.tile([128, NKB, HPC, DH + 1], DT)  # + ones column
            wq_sb = persist.tile([128, 8, HID], DT)
            wk_sb = persist.tile([128, 8, HID], DT)
            wv_sb = persist.tile([128, 8, HID], DT)
            wo_sb = persist.tile([64, HPC, D], DT)      # [dh, head, dcol]
            bq_sb = persist.tile([128, 4], F32)
            bk_sb = persist.tile([128, 4], F32)
            ones_sb = persist.tile([128, DH], F32)

            nc.sync.dma_start(out=wq_sb, in_=wq_d.rearrange("(kt p) n -> p kt n", p=128))
            nc.sync.dma_start(out=wk_sb, in_=wk_d.rearrange("(kt p) n -> p kt n", p=128))
            nc.sync.dma_start(out=wv_sb, in_=wv_d.rearrange("(kt p) n -> p kt n", p=128))
            nc.sync.dma_start(out=wo_sb, in_=wo_d.rearrange("(h p) n -> p h n", p=64))
            nc.sync.dma_start(out=bq_sb, in_=bq_d.rearrange("(h p) -> p h", p=128))
            nc.sync.dma_start(out=bk_sb, in_=bk_d.rearrange("(h p) -> p h", p=128))
            mask_sb = persist.tile([128, 2, QT], DT)
            nc.vector.memset(ones_sb, 1.0)
            nc.vector.memset(v_sb[:, :, :, DH : DH + 1], 1.0)
            nc.gpsimd.memset(mask_sb, 1.0)
            nc.gpsimd.affine_select(
                out=mask_sb, in_=mask_sb,
                compare_op=mybir.AluOpType.is_ge, fill=0.0, base=0,
                pattern=[[-128, 2], [1, QT]], channel_multiplier=-1,
            )

            def load_xt():
                nc.sync.dma_start(out=xT, in_=xt_d.rearrange("(kt p) t -> p kt t", p=128))

            def phase_C():
                # projections
                with tc.tile_pool(name="prj", bufs=2, space="PSUM") as prj_pool:
                    # qT / kT: psum [dcol 128, tok 512]; W tile stationary across
                    # the 4 token chunks (K-contiguous, one ldweights per 4 MMs)
                    for w_sb, b_sb, dst in ((wq_sb, bq_sb, qT), (wk_sb, bk_sb, kT)):
                        for p in range(4):
                            pss = [prj_pool.tile([128, 512], F32, tag=f"prj{ch}", name=f"prj{ch}")
                                   for ch in range(4)]
                            for kt in range(8):
                                for ch in range(4):
                                    nc.tensor.matmul(
                                        pss[ch],
                                        lhsT=w_sb[:, kt, p * 128 : (p + 1) * 128],
                                        rhs=xT[:, kt, ch * 512 : (ch + 1) * 512],
                                        start=(kt == 0),
                                        stop=(kt == 7),
                                    )
                            for ch in range(4):
                                nc.vector.tensor_scalar_add(
                                    out=dst[:, p, ch * 512 : (ch + 1) * 512],
                                    in0=pss[ch],
                                    scalar1=b_sb[:, p : p + 1],
                                )
                    # v: psum [tok 128, dcol 512]
                    for tt in range(NKB):
                        ps = prj_pool.tile([128, 512], F32, tag="prj0")
                        for kt in range(8):
                            nc.tensor.matmul(
                                ps,
                                lhsT=xT[:, kt, tt * 128 : (tt + 1) * 128],
                                rhs=wv_sb[:, kt, :],
                                start=(kt == 0),
                                stop=(kt == 7),
                            )
                        nc.vector.tensor_copy(
                            out=v_sb[:, tt, :, 0:DH],
                            in_=ps.rearrange("p (h d) -> p h d", h=HPC),
                        )

            def phase_D():
                # attention + output projection
                with (
                    tc.tile_pool(name="spsum", bufs=2, space="PSUM") as s_pool,
                    tc.tile_pool(name="acc", bufs=2, space="PSUM") as acc_pool,
                    tc.tile_pool(name="bc", bufs=1, space="PSUM") as bc_pool,
                    tc.tile_pool(name="opj", bufs=1, space="PSUM") as opj_pool,
                    tc.tile_pool(name="esc", bufs=3) as esc_pool,
                    tc.tile_pool(name="lrow", bufs=4) as lrow_pool,
                    tc.tile_pool(name="att", bufs=2) as att_pool,
                    tc.tile_pool(name="osb", bufs=2) as osb_pool,
                ):
                    for i in range(NI):
                        attnT = att_pool.tile([64, HPC, QT], DT, tag="attnT")
                        qs = slice(i * QT, (i + 1) * QT)
                        nj = 2 * i + 2   # number of 128-token key blocks
                        for pair in range(4):
                            accs = [acc_pool.tile([65, QT], F32, tag="acc", name=f"acc{h2}")
                                    for h2 in range(2)]
                            # groups of up to 4 key blocks share one psum/exp;
                            # heads interleave inside each group so one head's
                            # ldweights hide under the other head's matmuls
                            for j0 in range(0, nj, 4):
                                ng = min(4, nj - j0)
                                escs = []
                                for h2 in range(2):
                                    hp = slice(h2 * 64, h2 * 64 + 64)
                                    sps = s_pool.tile([128, 4, QT], F32, tag=f"s{h2}",
                                                      name=f"s{h2}", bufs=1)
                                    for jj in range(ng):
                                        j = j0 + jj
                                        nc.tensor.matmul(
                                            sps[:, jj, :],
                                            lhsT=kT[hp, pair, j * 128 : (j + 1) * 128],
                                            rhs=qT[hp, pair, qs],
                                            start=True,
                                            stop=True,
                                        )
                                    esc = esc_pool.tile([128, 4, QT], DT, tag=f"esc{h2}",
                                                        name=f"esc{h2}", bufs=4)
                                    nc.scalar.activation(
                                        out=esc[:, 0:ng, :], in_=sps[:, 0:ng, :],
                                        func=mybir.ActivationFunctionType.Exp,
                                        scale=0.125,
                                    )
                                    if j0 + ng == nj:
                                        # last group holds the 2 diagonal blocks:
                                        # zero esc[k, jj, q] where jj*128+k > q
                                        nc.gpsimd.affine_select(
                                            out=esc[:, ng - 2 : ng, :],
                                            in_=esc[:, ng - 2 : ng, :],
                                            compare_op=mybir.AluOpType.is_ge,
                                            fill=0.0,
                                            base=0,
                                            pattern=[[-128, 2], [1, QT]],
                                            channel_multiplier=-1,
                                        )
                                    escs.append(esc)
                                for h2 in range(2):
                                    head = 2 * pair + h2
                                    for jj in range(ng):
                                        j = j0 + jj
                                        nc.tensor.matmul(
                                            accs[h2],
                                            lhsT=v_sb[:, j, head, :],
                                            rhs=escs[h2][:, jj, :],
                                            start=(j == 0),
                                            stop=(j == nj - 1),
                                        )
                            # normalize: attnT[dh, q] = acc[0:64] * (1 / acc[64])
                            r64 = lrow_pool.tile([65, 2, QT], F32, tag="r64")
                            recip = lrow_pool.tile([1, 2, QT], F32, tag="recip")
                            bc = bc_pool.tile([64, 2, QT], F32, tag="bc")
                            bc_sb = lrow_pool.tile([64, 2, QT], F32, tag="bc_sb")
                            for h2 in range(2):
                                head = 2 * pair + h2
                                # reciprocal of the L row in-lane (partition 64),
                                # then shift to partition 0 with a tiny SBUF DMA
                                nc.vector.reciprocal(
                                    out=r64[64:65, h2, :], in_=accs[h2][64:65, :]
                                )
                                nc.sync.dma_start(
                                    out=recip[:, h2, :], in_=r64[64:65, h2, :]
                                )
                                nc.tensor.matmul(
                                    bc[:, h2, :],
                                    lhsT=ones_sb[0:1, :],
                                    rhs=recip[:, h2, :],
                                    start=True,
                                    stop=True,
                                )
                                nc.vector.tensor_copy(out=bc_sb[:, h2, :], in_=bc[:, h2, :])
                                nc.vector.tensor_mul(
                                    attnT[:, head, :], accs[h2][0:64, :], bc_sb[:, h2, :]
                                )
                        if "O" not in phases:
                            nc.gpsimd.dma_start(out=out_d[i * QT : i * QT + 64, 0:QT],
                                                in_=attnT[:, 0, :])
                        # output projection: contraction over 8 heads x 64 dh
                        for qc in range(2 if "O" in phases else 0):
                            osb = osb_pool.tile([128, D], F32, tag="osb")
                            for nch in range(2):
                                ops = opj_pool.tile([128, 512], F32, tag="opj")
                                for head in range(HPC):
                                    nc.tensor.matmul(
                                        ops,
                                        lhsT=attnT[:, head, qc * 128 : (qc + 1) * 128],
                                        rhs=wo_sb[:, head, nch * 512 : (nch + 1) * 512],
                                        start=(head == 0),
                                        stop=(head == HPC - 1),
                                    )
                                nc.vector.tensor_copy(
                                    out=osb[:, nch * 512 : (nch + 1) * 512], in_=ops
                                )
                            r0 = i * QT + qc * 128
                            nc.sync.dma_start(out=out_d[r0 : r0 + 128, :], in_=osb)

            def body():
                load_xt()
                if "C" in phases:
                    phase_C()
                if "D" in phases:
                    phase_D()
                # keep-alive DMAs for truncated variants (defeat DCE)
                if "D" not in phases:
                    nc.gpsimd.dma_start(out=out_d[0:128, :], in_=xT[:, 0, 0:D])
                    if "C" in phases:
                        nc.gpsimd.dma_start(out=out_d[128:256, :], in_=qT[:, 0, 0:D])
                        nc.gpsimd.dma_start(out=out_d[256:384, :], in_=kT[:, 0, 0:D])
                        nc.gpsimd.dma_start(out=out_d[384:512, 0:520], in_=v_sb[:, 0, :, :])

            if loop_n is None:
                body()
            else:
                with tc.For_i(0, loop_n, 1):
                    body()

    nc.compile()
    return nc


def get_nc(loop_n=None, phases="CDO"):
    key = ("nc", loop_n, phases)
    if key not in _CACHE:
        _CACHE[key] = _build_nc(loop_n, phases)
    return _CACHE[key]


def make_inputs(x, Wq, bq, Wk, bk, Wv, bv, Wo, bo):
    """Build the 8 per-core input maps (host-side sharding + x transpose)."""
    x = np.asarray(x, dtype=np.float32)
    wq_g = [np.ascontiguousarray(np.asarray(Wq)[:, g * HID : (g + 1) * HID]).astype(NPDT) for g in range(2)]
    wk_g = [np.ascontiguousarray(np.asarray(Wk)[:, g * HID : (g + 1) * HID]).astype(NPDT) for g in range(2)]
    wv_g = [np.ascontiguousarray(np.asarray(Wv)[:, g * HID : (g + 1) * HID]).astype(NPDT) for g in range(2)]
    wo_g = [np.ascontiguousarray(np.asarray(Wo)[g * HID : (g + 1) * HID, :]).astype(NPDT) for g in range(2)]
    bq_g = [np.ascontiguousarray(np.asarray(bq, dtype=np.float32)[g * HID : (g + 1) * HID]) for g in range(2)]
    bk_g = [np.ascontiguousarray(np.asarray(bk, dtype=np.float32)[g * HID : (g + 1) * HID]) for g in range(2)]
    xt_b = [np.ascontiguousarray(x[b].T).astype(NPDT) for b in range(B)]
    in_maps = []
    for c in range(8):
        b, g = c // 2, c % 2
        in_maps.append({
            "xt": xt_b[b], "wq": wq_g[g], "wk": wk_g[g], "wv": wv_g[g],
            "wo": wo_g[g], "bq": bq_g[g], "bk": bk_g[g],
        })
    return in_maps


def assemble(results, Wv_bias_term):
    out = np.empty((B, S, D), dtype=np.float32)
    for b in range(B):
        out[b] = results[2 * b]["out"] + results[2 * b + 1]["out"] + Wv_bias_term
    return out


def kernel(x, Wq, bq, Wk, bk, Wv, bv, Wo, bo):
    nc = get_nc()
    in_maps = make_inputs(x, Wq, bq, Wk, bk, Wv, bv, Wo, bo)
    res = run_bass_kernel_spmd(nc, in_maps, core_ids=list(range(8)))
    corr = (np.asarray(bv, dtype=np.float32) @ np.asarray(Wo, dtype=np.float32)
            + np.asarray(bo, dtype=np.float32))
    return assemble(res.results, corr)


# revision 30
# speedup vs baseline: 111.6687x; 1.1393x over previous
"""Trainium2 Bass kernel: causal multi-head self-attention (B=4, S=2048, D=1024, H=16).

Sharding (8 cores): core c -> batch b = c//2, head-group g = c%2 (8 heads each).
Each core computes softmax((x_b Wq_g)(x_b Wk_g)^T / sqrt(dh), causal) (x_b Wv_g) Wo_g
-> a partial [S, D] output.  Host sums the two head-group partials per batch and
adds the row-constant correction bo + bv @ Wo (valid because softmax rows sum to 1).

On-core pipeline (straight-line Tile kernel, bf16 matmuls / f32 accumulation):
  1. x arrives pre-transposed from the host as xT [feature, token]
  2. QKV projections: qT/kT in [dcol, token] layout (weight tiles kept stationary
     across 4 token chunks), v natural [token, dcol] with a ones-column appended
     (gives the softmax denominator for free in the AV matmul)
  3. flash-style causal attention in sT=[k,q] layout: scores MM (heads row-packed on
     the PE array), exp on ScalarE in 4-key-block batches (scores are O(+-6) so no
     max subtraction needed), multiplicative triangular mask on the diagonal blocks
     applied on the idle GPSIMD engine, AV-matmul accumulation
  4. normalize via reciprocal of the ones-column row + PE broadcast, then the
     output projection (contraction over this core's 512 hidden dims)
"""

import numpy as np
import ml_dtypes

import concourse.bass as bass
import concourse.mybir as mybir
import concourse.tile as tile
from concourse import bacc
from concourse.bass_utils import run_bass_kernel_spmd

B, S, D, H = 4, 2048, 1024, 16
DH = D // H            # 64
HPC = 8                # heads per core
HID = HPC * DH         # 512 hidden dims per core
QT = 512               # query mega-tile
NI = S // QT           # 8 query mega-tiles
NKB = S // 128         # 16 key blocks
F32 = mybir.dt.float32

# compute dtype for matmuls (bf16 = full-rate PE; f32 = half-rate, exact)
DT = mybir.dt.bfloat16
NPDT = ml_dtypes.bfloat16

_CACHE = {}


def _build_nc(loop_n=None, phases="CDO"):
    nc = bacc.Bacc("TRN2", target_bir_lowering=False, debug=False)

    xt_d = nc.dram_tensor("xt", [D, S], DT, kind="ExternalInput")   # host-transposed
    wq_d = nc.dram_tensor("wq", [D, HID], DT, kind="ExternalInput")
    wk_d = nc.dram_tensor("wk", [D, HID], DT, kind="ExternalInput")
    wv_d = nc.dram_tensor("wv", [D, HID], DT, kind="ExternalInput")
    wo_d = nc.dram_tensor("wo", [HID, D], DT, kind="ExternalInput")
    bq_d = nc.dram_tensor("bq", [HID], F32, kind="ExternalInput")
    bk_d = nc.dram_tensor("bk", [HID], F32, kind="ExternalInput")
    out_d = nc.dram_tensor("out", [S, D], F32, kind="ExternalOutput")

    with tile.TileContext(nc) as tc:
        with tc.tile_pool(name="persist", bufs=1) as persist:
            # persistent SBUF tensors
            xT = persist.tile([128, 8, S], DT)          # xT[p, kt, t] = x[t, kt*128+p]
            qT = persist.tile([128, 4, S], DT)          # [dh-in-pair, pair, token]
            kT = persist.tile([128, 4, S], DT)
            v_sb = persist.tile([128, NKB, HPC, DH + 1], DT)  # + ones column
            wq_sb = persist.tile([128, 8, HID], DT)
            wk_sb = persist.tile([128, 8, HID], DT)
            wv_sb = persist.tile([128, 8, HID], DT)
            wo_sb = persist.tile([64, HPC, D], DT)      # [dh, head, dcol]
            bq_sb = persist.tile([128, 4], F32)
            bk_sb = persist.tile([128, 4], F32)
            ones_sb = persist.tile([128, DH], F32)

            nc.sync.dma_start(out=wq_sb, in_=wq_d.rearrange("(kt p) n -> p kt n", p=128))
            nc.sync.dma_start(out=wk_sb, in_=wk_d.rearrange("(kt p) n -> p kt n", p=128))
            nc.sync.dma_start(out=wv_sb, in_=wv_d.rearrange("(kt p) n -> p kt n", p=128))
            nc.sync.dma_start(out=wo_sb, in_=wo_d.rearrange("(h p) n -> p h n", p=64))
            nc.sync.dma_start(out=bq_sb, in_=bq_d.rearrange("(h p) -> p h", p=128))
            nc.sync.dma_start(out=bk_sb, in_=bk_d.rearrange("(h p) -> p h", p=128))
            nc.vector.memset(ones_sb, 1.0)
            nc.vector.memset(v_sb[:, :, :, DH : DH + 1], 1.0)

            def load_xt():
                nc.sync.dma_start(out=xT, in_=xt_d.rearrange("(kt p) t -> p kt t", p=128))

            def phase_C():
                # projections
                with tc.tile_pool(name="prj", bufs=2, space="PSUM") as prj_pool:
                    # qT / kT: psum [dcol 128, tok 512]; W tile stationary across
                    # the 4 token chunks (K-contiguous, one ldweights per 4 MMs)
                    for w_sb, b_sb, dst in ((wq_sb, bq_sb, qT), (wk_sb, bk_sb, kT)):
                        for p in range(4):
                            pss = [prj_pool.tile([128, 512], F32, tag=f"prj{ch}", name=f"prj{ch}")
                                   for ch in range(4)]
                            for kt in range(8):
                                for ch in range(4):
                                    nc.tensor.matmul(
                                        pss[ch],
                                        lhsT=w_sb[:, kt, p * 128 : (p + 1) * 128],
                                        rhs=xT[:, kt, ch * 512 : (ch + 1) * 512],
                                        start=(kt == 0),
                                        stop=(kt == 7),
                                    )
                            for ch in range(4):
                                nc.vector.tensor_scalar_add(
                                    out=dst[:, p, ch * 512 : (ch + 1) * 512],
                                    in0=pss[ch],
                                    scalar1=b_sb[:, p : p + 1],
                                )
                    # v: psum [tok 128, dcol 512]
                    for tt in range(NKB):
                        ps = prj_pool.tile([128, 512], F32, tag="prj0")
                        for kt in range(8):
                            nc.tensor.matmul(
                                ps,
                                lhsT=xT[:, kt, tt * 128 : (tt + 1) * 128],
                                rhs=wv_sb[:, kt, :],
                                start=(kt == 0),
                                stop=(kt == 7),
                            )
                        nc.vector.tensor_copy(
                            out=v_sb[:, tt, :, 0:DH],
                            in_=ps.rearrange("p (h d) -> p h d", h=HPC),
                        )

            def phase_D():
                # attention + output projection
                with (
                    tc.tile_pool(name="spsum", bufs=2, space="PSUM") as s_pool,
                    tc.tile_pool(name="acc", bufs=2, space="PSUM") as acc_pool,
                    tc.tile_pool(name="bc", bufs=1, space="PSUM") as bc_pool,
                    tc.tile_pool(name="opj", bufs=1, space="PSUM") as opj_pool,
                    tc.tile_pool(name="esc", bufs=3) as esc_pool,
                    tc.tile_pool(name="lrow", bufs=2) as lrow_pool,
                    tc.tile_pool(name="att", bufs=2) as att_pool,
                    tc.tile_pool(name="osb", bufs=2) as osb_pool,
                ):
                    for i in range(NI):
                        attnT = att_pool.tile([64, HPC, QT], DT, tag="attnT")
                        qs = slice(i * QT, (i + 1) * QT)
                        nj = (i + 1) * (QT // 128)   # number of 128-token key blocks
                        for pair in range(4):
                            accs = [acc_pool.tile([65, QT], F32, tag="acc", name=f"acc{h2}")
                                    for h2 in range(2)]
                            # groups of up to 4 key blocks share one psum/exp;
                            # heads interleave inside each group so one head's
                            # ldweights hide under the other head's matmuls
                            for j0 in range(0, nj, 2):
                                ng = min(2, nj - j0)
                                escs = []
                                for h2 in range(2):
                                    hp = slice(h2 * 64, h2 * 64 + 64)
                                    sps = s_pool.tile([128, 2, QT], F32, tag=f"s{h2}",
                                                      name=f"s{h2}", bufs=1)
                                    for jj in range(ng):
                                        j = j0 + jj
                                        nc.tensor.matmul(
                                            sps[:, jj, :],
                                            lhsT=kT[hp, pair, j * 128 : (j + 1) * 128],
                                            rhs=qT[hp, pair, qs],
                                            start=True,
                                            stop=True,
                                        )
                                    esc = esc_pool.tile([128, 2, QT], DT, tag=f"esc{h2}",
                                                        name=f"esc{h2}", bufs=4)
                                    nc.scalar.activation(
                                        out=esc[:, 0:ng, :], in_=sps[:, 0:ng, :],
                                        func=mybir.ActivationFunctionType.Exp,
                                        scale=0.125,
                                    )
                                    band = nj - QT // 128  # first diagonal block
                                    if j0 + ng > band:
                                        # zero esc[k, jj, q] where (j-band)*128+k > q
                                        jj0 = max(0, band - j0)
                                        nsel = ng - jj0
                                        nc.gpsimd.affine_select(
                                            out=esc[:, jj0:ng, :],
                                            in_=esc[:, jj0:ng, :],
                                            compare_op=mybir.AluOpType.is_ge,
                                            fill=0.0,
                                            base=-128 * (j0 + jj0 - band),
                                            pattern=[[-128, nsel], [1, QT]],
                                            channel_multiplier=-1,
                                        )
                                    escs.append(esc)
                                for h2 in range(2):
                                    head = 2 * pair + h2
                                    for jj in range(ng):
                                        j = j0 + jj
                                        nc.tensor.matmul(
                                            accs[h2],
                                            lhsT=v_sb[:, j, head, :],
                                            rhs=escs[h2][:, jj, :],
                                            start=(j == 0),
                                            stop=(j == nj - 1),
                                        )
                            # normalize: attnT[dh, q] = acc[0:64] * (1 / acc[64])
                            r64 = lrow_pool.tile([65, 2, QT], F32, tag="r64")
                            recip = lrow_pool.tile([1, 2, QT], F32, tag="recip")
                            bcs = [bc_pool.tile([64, QT], F32, tag="bc", name=f"bc{h2}") for h2 in range(2)]
                            bc_sb = lrow_pool.tile([64, 2, QT], F32, tag="bc_sb")
                            for h2 in range(2):
                                head = 2 * pair + h2
                                # reciprocal of the L row in-lane (partition 64),
                                # then shift to partition 0 with a tiny SBUF DMA
                                nc.vector.reciprocal(
                                    out=r64[64:65, h2, :], in_=accs[h2][64:65, :]
                                )
                                nc.sync.dma_start(
                                    out=recip[:, h2, :], in_=r64[64:65, h2, :]
                                )
                                nc.tensor.matmul(
                                    bcs[h2],
                                    lhsT=ones_sb[0:1, :],
                                    rhs=recip[:, h2, :],
                                    start=True,
                                    stop=True,
                                )
                                nc.vector.tensor_copy(out=bc_sb[:, h2, :], in_=bcs[h2])
                                nc.vector.tensor_mul(
                                    attnT[:, head, :], accs[h2][0:64, :], bc_sb[:, h2, :]
                                )
                        if "O" not in phases:
                            nc.gpsimd.dma_start(out=out_d[i * QT : i * QT + 64, 0:QT],
                                                in_=attnT[:, 0, :])
                        # output projection: contraction over 8 heads x 64 dh
                        for qc in range(QT // 128 if "O" in phases else 0):
                            osb = osb_pool.tile([128, D], F32, tag="osb")
                            for nch in range(2):
                                ops = opj_pool.tile([128, 512], F32, tag="opj")
                                for head in range(HPC):
                                    nc.tensor.matmul(
                                        ops,
                                        lhsT=attnT[:, head, qc * 128 : (qc + 1) * 128],
                                        rhs=wo_sb[:, head, nch * 512 : (nch + 1) * 512],
                                        start=(head == 0),
                                        stop=(head == HPC - 1),
                                    )
                                nc.vector.tensor_copy(
                                    out=osb[:, nch * 512 : (nch + 1) * 512], in_=ops
                                )
                            r0 = i * QT + qc * 128
                            nc.sync.dma_start(out=out_d[r0 : r0 + 128, :], in_=osb)

            def body():
                load_xt()
                if "C" in phases:
                    phase_C()
                if "D" in phases:
                    phase_D()
                # keep-alive DMAs for truncated variants (defeat DCE)
                if "D" not in phases:
                    nc.gpsimd.dma_start(out=out_d[0:128, :], in_=xT[:, 0, 0:D])
                    if "C" in phases:
                        nc.gpsimd.dma_start(out=out_d[128:256, :], in_=qT[:, 0, 0:D])
                        nc.gpsimd.dma_start(out=out_d[256:384, :], in_=kT[:, 0, 0:D])
                        nc.gpsimd.dma_start(out=out_d[384:512, 0:520], in_=v_sb[:, 0, :, :])

            if loop_n is None:
                body()
            else:
                with tc.For_i(0, loop_n, 1):
                    body()

    nc.compile()
    return nc


def get_nc(loop_n=None, phases="CDO"):
    key = ("nc", loop_n, phases)
    if key not in _CACHE:
        _CACHE[key] = _build_nc(loop_n, phases)
    return _CACHE[key]


def make_inputs(x, Wq, bq, Wk, bk, Wv, bv, Wo, bo):
    """Build the 8 per-core input maps (host-side sharding + x transpose)."""
    x = np.asarray(x, dtype=np.float32)
    wq_g = [np.ascontiguousarray(np.asarray(Wq)[:, g * HID : (g + 1) * HID]).astype(NPDT) for g in range(2)]
    wk_g = [np.ascontiguousarray(np.asarray(Wk)[:, g * HID : (g + 1) * HID]).astype(NPDT) for g in range(2)]
    wv_g = [np.ascontiguousarray(np.asarray(Wv)[:, g * HID : (g + 1) * HID]).astype(NPDT) for g in range(2)]
    wo_g = [np.ascontiguousarray(np.asarray(Wo)[g * HID : (g + 1) * HID, :]).astype(NPDT) for g in range(2)]
    bq_g = [np.ascontiguousarray(np.asarray(bq, dtype=np.float32)[g * HID : (g + 1) * HID]) for g in range(2)]
    bk_g = [np.ascontiguousarray(np.asarray(bk, dtype=np.float32)[g * HID : (g + 1) * HID]) for g in range(2)]
    xt_b = [np.ascontiguousarray(x[b].T).astype(NPDT) for b in range(B)]
    in_maps = []
    for c in range(8):
        b, g = c // 2, c % 2
        in_maps.append({
            "xt": xt_b[b], "wq": wq_g[g], "wk": wk_g[g], "wv": wv_g[g],
            "wo": wo_g[g], "bq": bq_g[g], "bk": bk_g[g],
        })
    return in_maps


def assemble(results, Wv_bias_term):
    out = np.empty((B, S, D), dtype=np.float32)
    for b in range(B):
        out[b] = results[2 * b]["out"] + results[2 * b + 1]["out"] + Wv_bias_term
    return out


def kernel(x, Wq, bq, Wk, bk, Wv, bv, Wo, bo):
    nc = get_nc()
    in_maps = make_inputs(x, Wq, bq, Wk, bk, Wv, bv, Wo, bo)
    res = run_bass_kernel_spmd(nc, in_maps, core_ids=list(range(8)))
    corr = (np.asarray(bv, dtype=np.float32) @ np.asarray(Wo, dtype=np.float32)
            + np.asarray(bo, dtype=np.float32))
    return assemble(res.results, corr)


# revision 34
# speedup vs baseline: 113.7495x; 1.0186x over previous
"""Trainium2 Bass kernel: causal multi-head self-attention (B=4, S=2048, D=1024, H=16).

Sharding (8 cores): core c -> batch b = c//2, head-group g = c%2 (8 heads each).
Each core computes softmax((x_b Wq_g)(x_b Wk_g)^T / sqrt(dh), causal) (x_b Wv_g) Wo_g
-> a partial [S, D] output.  Host sums the two head-group partials per batch and
adds the row-constant correction bo + bv @ Wo (valid because softmax rows sum to 1).

On-core pipeline (straight-line Tile kernel, bf16 matmuls / f32 accumulation):
  1. x arrives pre-transposed from the host as xT [feature, token]
  2. QKV projections: qT/kT in [dcol, token] layout (weight tiles kept stationary
     across 4 token chunks), v natural [token, dcol] with a ones-column appended
     (gives the softmax denominator for free in the AV matmul)
  3. flash-style causal attention in sT=[k,q] layout: scores MM (heads row-packed on
     the PE array), exp on ScalarE in 4-key-block batches (scores are O(+-6) so no
     max subtraction needed), multiplicative triangular mask on the diagonal blocks
     applied on the idle GPSIMD engine, AV-matmul accumulation
  4. normalize via reciprocal of the ones-column row + PE broadcast, then the
     output projection (contraction over this core's 512 hidden dims)
"""

import numpy as np
import ml_dtypes

import concourse.bass as bass
import concourse.mybir as mybir
import concourse.tile as tile
from concourse import bacc
from concourse.bass_utils import run_bass_kernel_spmd

B, S, D, H = 4, 2048, 1024, 16
DH = D // H            # 64
HPC = 8                # heads per core
HID = HPC * DH         # 512 hidden dims per core
QT = 512               # query mega-tile
NI = S // QT           # 8 query mega-tiles
NKB = S // 128         # 16 key blocks
F32 = mybir.dt.float32

# compute dtype for matmuls (bf16 = full-rate PE; f32 = half-rate, exact)
DT = mybir.dt.bfloat16
NPDT = ml_dtypes.bfloat16

_CACHE = {}


def _build_nc(loop_n=None, phases="CDO"):
    nc = bacc.Bacc("TRN2", target_bir_lowering=False, debug=False)

    xt_d = nc.dram_tensor("xt", [D, S], DT, kind="ExternalInput")   # host-transposed
    wq_d = nc.dram_tensor("wq", [D, HID], DT, kind="ExternalInput")
    wk_d = nc.dram_tensor("wk", [D, HID], DT, kind="ExternalInput")
    wv_d = nc.dram_tensor("wv", [D, HID], DT, kind="ExternalInput")
    wo_d = nc.dram_tensor("wo", [HID, D], DT, kind="ExternalInput")
    bq_d = nc.dram_tensor("bq", [HID], F32, kind="ExternalInput")
    bk_d = nc.dram_tensor("bk", [HID], F32, kind="ExternalInput")
    out_d = nc.dram_tensor("out", [S, D], F32, kind="ExternalOutput")

    with tile.TileContext(nc) as tc:
        with tc.tile_pool(name="persist", bufs=1) as persist:
            # persistent SBUF tensors
            xT = persist.tile([128, 8, S], DT)          # xT[p, kt, t] = x[t, kt*128+p]
            qT = persist.tile([128, 4, S], DT)          # [dh-in-pair, pair, token]
            kT = persist.tile([128, 4, S], DT)
            v_sb = persist.tile([128, NKB, HPC, DH + 1], DT)  # + ones column
            wq_sb = persist.tile([128, 8, HID], DT)
            wk_sb = persist.tile([128, 8, HID], DT)
            wv_sb = persist.tile([128, 8, HID], DT)
            wo_sb = persist.tile([64, HPC, D], DT)      # [dh, head, dcol]
            bq_sb = persist.tile([128, 4], F32)
            bk_sb = persist.tile([128, 4], F32)
            ones_sb = persist.tile([128, DH], F32)

            nc.sync.dma_start(out=wq_sb, in_=wq_d.rearrange("(kt p) n -> p kt n", p=128))
            nc.sync.dma_start(out=wk_sb, in_=wk_d.rearrange("(kt p) n -> p kt n", p=128))
            nc.sync.dma_start(out=wv_sb, in_=wv_d.rearrange("(kt p) n -> p kt n", p=128))
            nc.sync.dma_start(out=wo_sb, in_=wo_d.rearrange("(h p) n -> p h n", p=64))
            nc.sync.dma_start(out=bq_sb, in_=bq_d.rearrange("(h p) -> p h", p=128))
            nc.sync.dma_start(out=bk_sb, in_=bk_d.rearrange("(h p) -> p h", p=128))
            nc.vector.memset(ones_sb, 1.0)
            nc.vector.memset(v_sb[:, :, :, DH : DH + 1], 1.0)

            def load_xt():
                # split by k-tile so the first projection matmuls start early
                xtv = xt_d.rearrange("(kt p) t -> p kt t", p=128)
                for kt in range(8):
                    nc.sync.dma_start(out=xT[:, kt, :], in_=xtv[:, kt, :])

            def phase_C():
                # projections
                with tc.tile_pool(name="prj", bufs=2, space="PSUM") as prj_pool:
                    # qT / kT: psum [dcol 128, tok 512]; W tile stationary across
                    # the 4 token chunks (K-contiguous, one ldweights per 4 MMs)
                    for w_sb, b_sb, dst in ((wq_sb, bq_sb, qT), (wk_sb, bk_sb, kT)):
                        for p in range(4):
                            pss = [prj_pool.tile([128, 512], F32, tag=f"prj{ch}", name=f"prj{ch}")
                                   for ch in range(4)]
                            for kt in range(8):
                                for ch in range(4):
                                    nc.tensor.matmul(
                                        pss[ch],
                                        lhsT=w_sb[:, kt, p * 128 : (p + 1) * 128],
                                        rhs=xT[:, kt, ch * 512 : (ch + 1) * 512],
                                        start=(kt == 0),
                                        stop=(kt == 7),
                                    )
                            for ch in range(4):
                                nc.vector.tensor_scalar_add(
                                    out=dst[:, p, ch * 512 : (ch + 1) * 512],
                                    in0=pss[ch],
                                    scalar1=b_sb[:, p : p + 1],
                                )
                    # v: psum [tok 128, dcol 512]
                    for tt in range(NKB):
                        ps = prj_pool.tile([128, 512], F32, tag="prj0")
                        for kt in range(8):
                            nc.tensor.matmul(
                                ps,
                                lhsT=xT[:, kt, tt * 128 : (tt + 1) * 128],
                                rhs=wv_sb[:, kt, :],
                                start=(kt == 0),
                                stop=(kt == 7),
                            )
                        nc.vector.tensor_copy(
                            out=v_sb[:, tt, :, 0:DH],
                            in_=ps.rearrange("p (h d) -> p h d", h=HPC),
                        )

            def phase_D():
                # attention + output projection
                with (
                    tc.tile_pool(name="spsum", bufs=2, space="PSUM") as s_pool,
                    tc.tile_pool(name="acc", bufs=2, space="PSUM") as acc_pool,
                    tc.tile_pool(name="bc", bufs=1, space="PSUM") as bc_pool,
                    tc.tile_pool(name="opj", bufs=1, space="PSUM") as opj_pool,
                    tc.tile_pool(name="esc", bufs=3) as esc_pool,
                    tc.tile_pool(name="lrow", bufs=2) as lrow_pool,
                    tc.tile_pool(name="att", bufs=2) as att_pool,
                    tc.tile_pool(name="osb", bufs=2) as osb_pool,
                ):
                    for i in range(NI):
                        attnT = att_pool.tile([64, HPC, QT], DT, tag="attnT")
                        qs = slice(i * QT, (i + 1) * QT)
                        nj = (i + 1) * (QT // 128)   # number of 128-token key blocks
                        for pair in range(4):
                            accs = [acc_pool.tile([65, QT], F32, tag="acc", name=f"acc{h2}")
                                    for h2 in range(2)]
                            # groups of up to 4 key blocks share one psum/exp;
                            # heads interleave inside each group so one head's
                            # ldweights hide under the other head's matmuls
                            for j0 in range(0, nj, 2):
                                ng = min(2, nj - j0)
                                escs = []
                                for h2 in range(2):
                                    hp = slice(h2 * 64, h2 * 64 + 64)
                                    sps = s_pool.tile([128, 2, QT], F32, tag=f"s{h2}",
                                                      name=f"s{h2}", bufs=1)
                                    for jj in range(ng):
                                        j = j0 + jj
                                        nc.tensor.matmul(
                                            sps[:, jj, :],
                                            lhsT=kT[hp, pair, j * 128 : (j + 1) * 128],
                                            rhs=qT[hp, pair, qs],
                                            start=True,
                                            stop=True,
                                        )
                                    esc = esc_pool.tile([128, 2, QT], DT, tag=f"esc{h2}",
                                                        name=f"esc{h2}", bufs=4)
                                    nc.scalar.activation(
                                        out=esc[:, 0:ng, :], in_=sps[:, 0:ng, :],
                                        func=mybir.ActivationFunctionType.Exp,
                                        scale=0.125,
                                    )
                                    band = nj - QT // 128  # first diagonal block
                                    for jj in range(max(0, band - j0), ng):
                                        # zero esc[k, jj, q] where (j-band)*128+k > q
                                        # (one select per block: its AV matmul only
                                        # waits for its own mask)
                                        nc.gpsimd.affine_select(
                                            out=esc[:, jj : jj + 1, :],
                                            in_=esc[:, jj : jj + 1, :],
                                            compare_op=mybir.AluOpType.is_ge,
                                            fill=0.0,
                                            base=-128 * (j0 + jj - band),
                                            pattern=[[-128, 1], [1, QT]],
                                            channel_multiplier=-1,
                                        )
                                    escs.append(esc)
                                for h2 in range(2):
                                    head = 2 * pair + h2
                                    for jj in range(ng):
                                        j = j0 + jj
                                        nc.tensor.matmul(
                                            accs[h2],
                                            lhsT=v_sb[:, j, head, :],
                                            rhs=escs[h2][:, jj, :],
                                            start=(j == 0),
                                            stop=(j == nj - 1),
                                        )
                            # normalize: attnT[dh, q] = acc[0:64] * (1 / acc[64])
                            r64 = lrow_pool.tile([65, 2, QT], F32, tag="r64")
                            recip = lrow_pool.tile([1, 2, QT], F32, tag="recip")
                            bcs = [bc_pool.tile([64, QT], F32, tag="bc", name=f"bc{h2}") for h2 in range(2)]
                            bc_sb = lrow_pool.tile([64, 2, QT], F32, tag="bc_sb")
                            for h2 in range(2):
                                head = 2 * pair + h2
                                # reciprocal of the L row in-lane (partition 64),
                                # then shift to partition 0 with a tiny SBUF DMA
                                nc.vector.reciprocal(
                                    out=r64[64:65, h2, :], in_=accs[h2][64:65, :]
                                )
                                nc.sync.dma_start(
                                    out=recip[:, h2, :], in_=r64[64:65, h2, :]
                                )
                                nc.tensor.matmul(
                                    bcs[h2],
                                    lhsT=ones_sb[0:1, :],
                                    rhs=recip[:, h2, :],
                                    start=True,
                                    stop=True,
                                )
                                nc.vector.tensor_copy(out=bc_sb[:, h2, :], in_=bcs[h2])
                                nc.vector.tensor_mul(
                                    attnT[:, head, :], accs[h2][0:64, :], bc_sb[:, h2, :]
                                )
                        if "O" not in phases:
                            nc.gpsimd.dma_start(out=out_d[i * QT : i * QT + 64, 0:QT],
                                                in_=attnT[:, 0, :])
                        # output projection: contraction over 8 heads x 64 dh
                        for qc in range(QT // 128 if "O" in phases else 0):
                            osb = osb_pool.tile([128, D], F32, tag="osb")
                            for nch in range(2):
                                ops = opj_pool.tile([128, 512], F32, tag="opj")
                                for head in range(HPC):
                                    nc.tensor.matmul(
                                        ops,
                                        lhsT=attnT[:, head, qc * 128 : (qc + 1) * 128],
                                        rhs=wo_sb[:, head, nch * 512 : (nch + 1) * 512],
                                        start=(head == 0),
                                        stop=(head == HPC - 1),
                                    )
                                nc.vector.tensor_copy(
                                    out=osb[:, nch * 512 : (nch + 1) * 512], in_=ops
                                )
                            r0 = i * QT + qc * 128
                            nc.sync.dma_start(out=out_d[r0 : r0 + 128, :], in_=osb)

            def body():
                load_xt()
                if "C" in phases:
                    phase_C()
                if "D" in phases:
                    phase_D()
                # keep-alive DMAs for truncated variants (defeat DCE)
                if "D" not in phases:
                    nc.gpsimd.dma_start(out=out_d[0:128, :], in_=xT[:, 0, 0:D])
                    if "C" in phases:
                        nc.gpsimd.dma_start(out=out_d[128:256, :], in_=qT[:, 0, 0:D])
                        nc.gpsimd.dma_start(out=out_d[256:384, :], in_=kT[:, 0, 0:D])
                        nc.gpsimd.dma_start(out=out_d[384:512, 0:520], in_=v_sb[:, 0, :, :])

            if loop_n is None:
                body()
            else:
                with tc.For_i(0, loop_n, 1):
                    body()

    nc.compile()
    return nc


def get_nc(loop_n=None, phases="CDO"):
    key = ("nc", loop_n, phases)
    if key not in _CACHE:
        _CACHE[key] = _build_nc(loop_n, phases)
    return _CACHE[key]


def make_inputs(x, Wq, bq, Wk, bk, Wv, bv, Wo, bo):
    """Build the 8 per-core input maps (host-side sharding + x transpose)."""
    x = np.asarray(x, dtype=np.float32)
    wq_g = [np.ascontiguousarray(np.asarray(Wq)[:, g * HID : (g + 1) * HID]).astype(NPDT) for g in range(2)]
    wk_g = [np.ascontiguousarray(np.asarray(Wk)[:, g * HID : (g + 1) * HID]).astype(NPDT) for g in range(2)]
    wv_g = [np.ascontiguousarray(np.asarray(Wv)[:, g * HID : (g + 1) * HID]).astype(NPDT) for g in range(2)]
    wo_g = [np.ascontiguousarray(np.asarray(Wo)[g * HID : (g + 1) * HID, :]).astype(NPDT) for g in range(2)]
    bq_g = [np.ascontiguousarray(np.asarray(bq, dtype=np.float32)[g * HID : (g + 1) * HID]) for g in range(2)]
    bk_g = [np.ascontiguousarray(np.asarray(bk, dtype=np.float32)[g * HID : (g + 1) * HID]) for g in range(2)]
    xt_b = [np.ascontiguousarray(x[b].T).astype(NPDT) for b in range(B)]
    in_maps = []
    for c in range(8):
        b, g = c // 2, c % 2
        in_maps.append({
            "xt": xt_b[b], "wq": wq_g[g], "wk": wk_g[g], "wv": wv_g[g],
            "wo": wo_g[g], "bq": bq_g[g], "bk": bk_g[g],
        })
    return in_maps


def assemble(results, Wv_bias_term):
    out = np.empty((B, S, D), dtype=np.float32)
    for b in range(B):
        out[b] = results[2 * b]["out"] + results[2 * b + 1]["out"] + Wv_bias_term
    return out


def kernel(x, Wq, bq, Wk, bk, Wv, bv, Wo, bo):
    nc = get_nc()
    in_maps = make_inputs(x, Wq, bq, Wk, bk, Wv, bv, Wo, bo)
    res = run_bass_kernel_spmd(nc, in_maps, core_ids=list(range(8)))
    corr = (np.asarray(bv, dtype=np.float32) @ np.asarray(Wo, dtype=np.float32)
            + np.asarray(bo, dtype=np.float32))
    return assemble(res.results, corr)
